# revision 40
# baseline (speedup 1.0000x reference)
"""Fused multi-head attention (B=2, N=2048, C=1024, H=16) on 8 TRN2 NeuronCores.

Sharding: core = (b, g) with b = batch (2) and g = head-group of 4 heads (4).
Each core computes, for its batch and 4 heads:
    qkv slice -> per-head softmax attention -> out-proj partial (row-parallel).
Host sums the 4 per-head-group proj partials per batch and adds b_proj.

Device algorithm (per core), matmuls in bf16 (default) or float32r (TF32):
  phase 1: qkT = (x @ Wqk)^T   [q/k feats on partitions, 2048 tokens]
           v   = x @ Wv        [2048 tokens, 4*64] (+ ones column per head)
  phase 2: per (head pair, 512-row chunk):
           S^T tiles = matmul(lhsT=kTp_h, rhs=q-chunk)  [128 keys, 512 rows]
             kTp is K=128 zero-padded per head (even head rows 0:64, odd
             64:128) so full-array matmuls select one head's contraction
           expST = exp(S^T/8)  (ScalarE, PSUM->SBUF, pairs of key chunks)
           outT[65, rows] += [v_h|1]^T-matmul expST  (K=128 keys)
             row 64 = softmax denominator (ones column trick)
           outT[0:64] *= 1/denominator  (DVE recip, GpSimd bcast, DVE mult)
  phase 3: partial = out^T-matmul Wp -> bf16 -> DMA out

Schedule: a minimal prologue (q+k heads01 nt0, v keys 0:256) starts attention
block (0,0) at ~7us; ALL remaining qkv work rides the fill queue inside the
blocks' PE slack, paced per kc2 step so kTp/v chunks land just before their
consumers (first exp at ~12us vs ~48us for a separate qkv phase).  Inputs are
token-major and split across the sync/gpsimd/scalar DMA queues (the ~650ns
dma_start queue cost gates the prologue); proj output stages two 512-col
halves into one [128,1024] bf16 tile so each token chunk is a single 2KB-row
DMA; proj fill-pops spread 1/step through the exp-paced blocks.  The final
block normalizes outT per 128-token chunk and launches that chunk's proj+DMA
immediately, with the denominator staged via the then-idle ScalarE so the PE
clock stays warm through the tail.

Totals per core: PE ~191us busy (the binding engine; streaming floor 164us),
ScalarE exp ~150us (128 ACTs of [128,1024]; PSUM's 8x2KB banks cannot fit
wider double-buffered ST tiles, bf16 matmul PSUM output is TRN3-only, and
DVE-staged SBUF exp regressed ~50us -- serialization beat the ACT savings),
DVE ~92us.  HW exec ~224us in the machine's fast state (baseline 253.7us =
separate-phase schedule + 4us-per-head iterative-divide reciprocal).
Note: the box drifts between "fast" and ~20% slower power states on minute
timescales; compare variants only via interleaved runs in one process
(bench.py).
"""

import os
from contextlib import ExitStack

import numpy as np

import concourse.bass as bass
import concourse.mybir as mybir
import concourse.tile as tile
from concourse import bacc
from concourse.bass_utils import run_bass_kernel_spmd

B, N, C = 2, 2048, 1024
HC = 4  # heads per core
D = 64
NCORES = 8
KC = C // 128  # 8 contraction chunks for phase 1
SCALE = D**-0.5  # 0.125

# "f32r" (fp32 data, full-rate PE mode), "bf16", or "f32" (4x slower PE)
MM_DT = os.environ.get("ATTN_MM_DT", "bf16")
ST_TILE_POS = os.environ.get("ATTN_ST_TILE_POS", "1") == "1"
ACT_COPY = os.environ.get("ATTN_ACT_COPY", "0") == "1"


def _np_in_dtype():
    if MM_DT == "bf16":
        import ml_dtypes

        return np.dtype(ml_dtypes.bfloat16)
    return np.dtype(np.float32)


def _prep(a):
    """Cast to the device input dtype; for f32r, pre-round to TF32 (RTNE)."""
    a = np.ascontiguousarray(a)
    if MM_DT != "f32r":
        return a.astype(_np_in_dtype())
    u = a.astype(np.float32).view(np.uint32)
    u = (u + 0x0FFF + ((u >> 13) & 1)) & np.uint32(0xFFFFE000)
    return u.view(np.float32)



def _copy(eng, out, in_):
    if hasattr(eng, "tensor_copy"):
        eng.tensor_copy(out, in_)
    else:
        eng.copy(out, in_)

def build_nc():
    f32 = mybir.dt.float32
    in_dt = {
        "bf16": mybir.dt.bfloat16,
        "f32r": mybir.dt.float32r,
        "f32": mybir.dt.float32,
    }[MM_DT]
    mm = lambda ap: ap  # noqa: E731

    out_dt = mybir.dt.bfloat16 if MM_DT == "bf16" else f32

    nc = bacc.Bacc("TRN2", target_bir_lowering=False, debug=False, num_devices=NCORES)
    xT_d = nc.dram_tensor("xT", [C, N], in_dt, kind="ExternalInput").ap()
    wqk_d = nc.dram_tensor("wqk", [C, 2 * HC * D], in_dt, kind="ExternalInput").ap()
    wv_d = nc.dram_tensor("wv", [C, HC * D], in_dt, kind="ExternalInput").ap()
    wp_d = nc.dram_tensor("wp", [HC * D, C], in_dt, kind="ExternalInput").ap()
    # bf16 proj partials: halves the output DMA (the tail's critical path);
    # the host accumulates the 4 partials per batch in f32.
    out_d = nc.dram_tensor("out", [N, C], out_dt, kind="ExternalOutput").ap()

    with tile.TileContext(nc) as tc:
        with (
            tc.tile_pool(name="const", bufs=1) as const,
            tc.tile_pool(name="ex", bufs=8) as expool,
            tc.tile_pool(name="den", bufs=6) as dpool,
            tc.tile_pool(name="stage", bufs=4) as stage,
            tc.tile_pool(name="stps", bufs=2, space="PSUM") as stps,
            tc.tile_pool(name="pvps", bufs=4, space="PSUM") as pvps,
        ):
            # persistent tiles
            # qkT chunks: 0 = q heads 0,1; 1 = q heads 2,3
            #   (head even -> partitions 0:64, odd -> 64:128)
            # kTp: per-head zero-padded K=128 stationary operand: head even
            #   has kT in rows 0:64 / zeros in 64:128, head odd the reverse,
            #   so a full-128-row matmul against the stacked q chunk
            #   contracts only the matching head's 64 features.
            qkT_sb = const.tile([128, 2, N], in_dt, tag="qkT")
            kTp_sb = const.tile([128, HC, N], in_dt, tag="kTp")
            v_sb = const.tile([128, 16, HC, D + 1], in_dt, tag="v")
            wp_sb = const.tile([128, 2, C], in_dt, tag="wp")
            outT_sb = const.tile([128, 2, N], in_dt, tag="outT")
            xT_sb = const.tile([128, KC, N], in_dt, tag="xT")
            wqk_sb = const.tile([128, KC, 2 * HC * D], in_dt, tag="wqk")
            wv_sb = const.tile([128, KC, HC * D], in_dt, tag="wv")

            # ---- DMAs, token-major for xT: wqk/xT(nt=0) interleaved so the
            # prologue qk chains start after ~0.5MB; attention block (0,0)
            # can then begin at ~6us instead of ~40us.
            # Inputs split across the Sync and GpSimd HW DMA queues so two
            # engines drain in parallel; the prologue-critical wqk+xT(nt=0)
            # chunks alternate queues kc-wise to land first.
            # The ~650ns/dma_start queue cost gates the prologue, so the 16
            # critical transfers (wqk + xT nt0) go 3-way across sync, gpsimd
            # AND the (otherwise idle at startup) scalar queue.
            qs = [nc.sync, nc.gpsimd, nc.scalar]

            def dma_xt(nt, q):
                for kc in range(KC):
                    q.dma_start(
                        xT_sb[:, kc, nt * 512 : (nt + 1) * 512],
                        xT_d[kc * 128 : (kc + 1) * 128, nt * 512 : (nt + 1) * 512],
                    )

            for kc in range(KC):
                qs[(2 * kc) % 3].dma_start(
                    wqk_sb[:, kc, :], wqk_d[kc * 128 : (kc + 1) * 128, :]
                )
                qs[(2 * kc + 1) % 3].dma_start(
                    xT_sb[:, kc, 0:512], xT_d[kc * 128 : (kc + 1) * 128, 0:512]
                )
            for kc in range(KC):
                qs[kc % 2].dma_start(wv_sb[:, kc, :], wv_d[kc * 128 : (kc + 1) * 128, :])
            dma_xt(1, nc.gpsimd)
            dma_xt(2, nc.sync)
            dma_xt(3, nc.gpsimd)
            for c2 in range(2):
                nc.sync.dma_start(wp_sb[:, c2, :], wp_d[c2 * 128 : (c2 + 1) * 128, :])

            # ---- one-time fills (run during the DMA wait) ----
            zsrc = const.tile([64, 512], f32, tag="zsrc")
            nc.vector.memset(zsrc[:], 0.0)
            for h in range(HC):
                zb = 64 if h % 2 == 0 else 0
                for nt in range(4):
                    nc.vector.tensor_copy(
                        kTp_sb[zb : zb + 64, h, nt * 512 : (nt + 1) * 512], zsrc[:]
                    )
            ones_f32 = const.tile([128, 16, HC, 1], f32, tag="ones")
            nc.vector.memset(ones_f32[:], 1.0)
            nc.vector.tensor_copy(v_sb[:, :, :, D : D + 1], ones_f32[:])

            # ---- emission helpers ----
            def qk_chunk(mf, nt):
                """One psum of (x @ Wqk)^T: feat chunk mf, token chunk nt.
                wqk feat chunks: 0 = q heads 0,1; 1 = q heads 2,3;
                2 = k heads 0,1; 3 = k heads 2,3."""
                ps = pvps.tile([128, 512], f32, tag="pv", name="pv")
                for kc in range(KC):
                    nc.tensor.matmul(
                        ps,
                        mm(wqk_sb[:, kc, mf * 128 : (mf + 1) * 128]),
                        mm(xT_sb[:, kc, nt * 512 : (nt + 1) * 512]),
                        start=(kc == 0),
                        stop=(kc == KC - 1),
                    )
                nts = slice(nt * 512, (nt + 1) * 512)
                if mf % 2 == 0:
                    mq = mf // 2
                    if mf < 2:
                        nc.vector.tensor_copy(qkT_sb[:, mq, nts], ps)
                    else:
                        pass
                if mf < 2:
                    if mf == 1:
                        nc.vector.tensor_copy(qkT_sb[:, 1, nts], ps)
                else:
                    h0, h1 = 2 * (mf - 2), 2 * (mf - 2) + 1
                    nc.vector.tensor_copy(kTp_sb[0:64, h0, nts], ps[0:64, :])
                    nc.vector.tensor_copy(kTp_sb[64:128, h1, nts], ps[64:128, :])

            def v_chunk(t):
                """One psum of v = x @ Wv for token(=key) chunk t, all heads."""
                ps = pvps.tile([128, 512], f32, tag="pv", name="pv")[:, : HC * D]
                for kc in range(KC):
                    nc.tensor.matmul(
                        ps,
                        mm(xT_sb[:, kc, t * 128 : (t + 1) * 128]),
                        mm(wv_sb[:, kc, :]),
                        start=(kc == 0),
                        stop=(kc == KC - 1),
                    )
                nc.vector.tensor_copy(
                    v_sb[:, t, :, 0:D], ps.rearrange("p (h d) -> p h d", h=HC)
                )

            sg2_of = {}

            def proj_chunk(t, nf):
                """partial[t*128:(t+1)*128, nf*512:(nf+1)*512] = out @ Wp.
                Both nf halves stage into one [128,1024] tile; the DMA (2KB
                rows, half the packets) fires once per token chunk."""
                ps = pvps.tile([128, 512], f32, tag="pv", name="pv")
                for c2 in range(2):
                    nc.tensor.matmul(
                        ps,
                        mm(outT_sb[:, c2, t * 128 : (t + 1) * 128]),
                        mm(wp_sb[:, c2, nf * 512 : (nf + 1) * 512]),
                        start=(c2 == 0),
                        stop=(c2 == 1),
                    )
                if nf == 0:
                    sg2_of[t] = stage.tile(
                        [128, 1024], out_dt, tag="sg2", name="sg2", bufs=2
                    )
                sg = sg2_of[t]
                nc.vector.tensor_copy(sg[:, nf * 512 : (nf + 1) * 512], ps)
                if nf == 1:
                    nc.sync.dma_start(out_d[t * 128 : (t + 1) * 128, :], sg)
                    del sg2_of[t]

            def proj_tail(t):
                """Both nf halves of token chunk t in one stps-pool psum
                (free after the last exp): fewer, wider tail ops + 2KB-row
                output DMA."""
                ps = stps.tile([128, 1024], f32, tag="st", name="st")
                for nf in range(2):
                    for c2 in range(2):
                        nc.tensor.matmul(
                            ps[:, nf * 512 : (nf + 1) * 512],
                            mm(outT_sb[:, c2, t * 128 : (t + 1) * 128]),
                            mm(wp_sb[:, c2, nf * 512 : (nf + 1) * 512]),
                            start=(c2 == 0),
                            stop=(c2 == 1),
                        )
                sg = stage.tile([128, 1024], out_dt, tag="sg2", name="sg2", bufs=2)
                nc.vector.tensor_copy(sg, ps)
                # tail runs after the last ACT, so the scalar queue is free
                (nc.sync if t % 2 == 0 else nc.scalar).dma_start(
                    out_d[t * 128 : (t + 1) * 128, :], sg
                )

            # fill queue: work interleaved into attention blocks' PE slack
            fills = []

            def attention_block(hp, rc, sched, tail=False):
                """ST + exp + PV for head pair hp, 512-row chunk rc; pops
                sched[kc2] fill closures at the top of each kc2 step."""
                heads = (2 * hp, 2 * hp + 1)
                pv = {
                    h: pvps.tile([128, 512], f32, tag="pv", name="pv") for h in heads
                }
                for kc2 in range(8):
                    for _ in range(sched[kc2]):
                        if fills:
                            fills.pop(0)()
                    stp = {
                        h: stps.tile([128, 1024], f32, tag="st", name="st")
                        for h in heads
                    }
                    for j in range(2):
                        kc = 2 * kc2 + j
                        for h in heads:
                            nc.tensor.matmul(
                                stp[h][:, j * 512 : (j + 1) * 512],
                                mm(kTp_sb[:, h, kc * 128 : (kc + 1) * 128]),
                                mm(qkT_sb[:, hp, rc * 512 : (rc + 1) * 512]),
                                start=True,
                                stop=True,
                            )
                    for h in heads:
                        ex = expool.tile([128, 1024], in_dt, tag="ex", name="ex")
                        nc.scalar.activation(
                            ex, stp[h], mybir.ActivationFunctionType.Exp, scale=SCALE
                        )
                        for j in range(2):
                            kc = 2 * kc2 + j
                            nc.tensor.matmul(
                                pv[h][: D + 1, :],
                                mm(v_sb[:, kc, h, :]),
                                mm(ex[:, j * 512 : (j + 1) * 512]),
                                start=(kc == 0),
                                stop=(kc == 15),
                            )
                # both recips first: DVE stays busy while GpSimd runs the
                # first broadcast, finishing the chain ~1us sooner (keeps the
                # post-block stall under the 3.4us HAM re-throttle window).
                # approx-fast recip: ~51 ULP, ~5x faster than the iterative
                # divide -- the denominator only needs ~1e-2 relative.
                dens, rbcs = {}, {}
                for h in heads:
                    dens[h] = dpool.tile([1, 512], f32, tag="den", name="den")
                    if os.environ.get("ATTN_RECIP", "fast") == "fast":
                        dsrc = dpool.tile([1, 512], f32, tag="dsrc", name="dsrc")
                        # tail: ScalarE is idle after the last exp -- staging
                        # the denominator there keeps the PE-idle gap under
                        # the ~3.4us HAM re-throttle window, so the tail proj
                        # matmuls run at full clock.
                        if tail:
                            nc.scalar.copy(dsrc, pv[h][D : D + 1, :])
                        else:
                            nc.vector.tensor_copy(dsrc, pv[h][D : D + 1, :])
                        nc.vector.reciprocal_approx_fast(out=dens[h], in_=dsrc)
                    else:
                        nc.vector.reciprocal(dens[h], pv[h][D : D + 1, :])
                for h in heads:
                    rbcs[h] = dpool.tile([64, 512], f32, tag="rbc", name="rbc")
                    nc.gpsimd.partition_broadcast(rbcs[h], dens[h])
                if not tail:
                    for h in heads:
                        hb = (h % 2) * 64
                        nc.vector.tensor_tensor(
                            out=outT_sb[hb : hb + 64, hp, rc * 512 : (rc + 1) * 512],
                            in0=pv[h][0:D, :],
                            in1=rbcs[h][:],
                            op=mybir.AluOpType.mult,
                        )
                else:
                    # final block: normalize per 128-token chunk and launch
                    # that chunk's out-proj + DMA immediately, so the tail
                    # pipeline (mult -> proj MM -> cast -> DMA) overlaps
                    # instead of serializing after the whole block.
                    for tc4 in range(4):
                        ts = slice(rc * 512 + tc4 * 128, rc * 512 + tc4 * 128 + 128)
                        for h in heads:
                            hb = (h % 2) * 64
                            nc.vector.tensor_tensor(
                                out=outT_sb[hb : hb + 64, hp, ts],
                                in0=pv[h][0:D, tc4 * 128 : (tc4 + 1) * 128],
                                in1=rbcs[h][:, tc4 * 128 : (tc4 + 1) * 128],
                                op=mybir.AluOpType.mult,
                            )
                        proj_tail(4 * rc + tc4)

            # ---- schedule ----
            def queue_proj(rc):
                fills.extend(
                    [
                        lambda t=t, nf=nf: proj_chunk(t, nf)
                        for t in range(4 * rc, 4 * rc + 4)
                        for nf in range(2)
                    ]
                )

            # Minimal prologue: exactly the chunks block (0,0) step 0 needs
            # (q heads01 rc0, kTp keys 0:256, v keys 0:256). Everything else
            # rides the fill queue inside the attention blocks, so ScalarE
            # exp starts at ~7us instead of ~40us and the PE is never
            # DMA-starved for long (HAM stays warm).
            qk_chunk(0, 0)
            qk_chunk(2, 0)
            v_chunk(0)
            v_chunk(1)
            # Fill order: v(2k),v(2k+1) land just before their PV(k) consumer
            # in block (0,0); kTp chunk qk(2,nt) before the step contracting
            # those keys; q chunks qk(0,1..3) before blocks (0,1..3).
            fills.extend(
                [
                    lambda: v_chunk(2),
                    lambda: v_chunk(3),
                    lambda: qk_chunk(2, 1),
                    lambda: v_chunk(4),
                    lambda: v_chunk(5),
                    lambda: v_chunk(6),
                    lambda: v_chunk(7),
                    lambda: qk_chunk(2, 2),
                    lambda: v_chunk(8),
                    lambda: v_chunk(9),
                    lambda: v_chunk(10),
                    lambda: v_chunk(11),
                    lambda: qk_chunk(2, 3),
                    lambda: v_chunk(12),
                    lambda: v_chunk(13),
                    lambda: qk_chunk(0, 1),
                    lambda: v_chunk(14),
                    lambda: v_chunk(15),
                ]
            )
            attention_block(0, 0, sched=[0, 3, 2, 3, 2, 3, 2, 3])
            fills.append(lambda: qk_chunk(0, 2))
            fills.append(lambda: qk_chunk(0, 3))
            fills.extend([lambda nt=nt: qk_chunk(1, nt) for nt in range(4)])
            fills.extend([lambda nt=nt: qk_chunk(3, nt) for nt in range(4)])
            # Alternating hp order: each (1,rc) block's denominator chain gets
            # a full unrelated block of latency before proj(rc) pops consume
            # its outT, so the PE never stalls on the recip/bcast/mult chain.
            late = [0, 1, 1, 1, 1, 1, 1, 1]
            attention_block(0, 1, sched=[2, 2, 2, 1, 1, 1, 1, 1])
            attention_block(1, 0, sched=[0] * 8)
            queue_proj(0)
            attention_block(0, 2, sched=[0] * 8)
            attention_block(1, 1, sched=late)
            queue_proj(1)
            attention_block(0, 3, sched=[0, 0, 0, 1, 1, 0, 0, 0])
            attention_block(1, 2, sched=late)
            queue_proj(2)
            attention_block(1, 3, sched=[0, 0, 0, 2, 2, 3, 1, 1], tail=True)
            # drain any straggler fills (none expected)
            while fills:
                fills.pop(0)()
    nc.compile()
    return nc


def make_in_maps(x, w_qkv, w_proj):
    in_maps = []
    for core in range(NCORES):
        b, g = core // 4, core % 4
        qs = slice(g * 256, (g + 1) * 256)
        in_maps.append(
            {
                "xT": _prep(x[b].T),
                "wqk": _prep(
                    np.concatenate(
                        [w_qkv[:, qs], w_qkv[:, C + g * 256 : C + (g + 1) * 256]],
                        axis=1,
                    )
                ),
                "wv": _prep(w_qkv[:, 2 * C + g * 256 : 2 * C + (g + 1) * 256]),
                "wp": _prep(w_proj[qs, :]),
            }
        )
    return in_maps


def run_hw(x, w_qkv, w_proj, b_proj, trace=False, tmpdir=None):
    """Returns (full output [2, 2048, 1024] f32, exec_time_ns or None)."""
    in_maps = make_in_maps(x, w_qkv, w_proj)
    nc = build_nc()
    r = run_bass_kernel_spmd(
        nc, in_maps, core_ids=list(range(NCORES)), trace=trace, tmpdir=tmpdir
    )
    full = np.zeros((B, N, C), np.float32)
    for core in range(NCORES):
        full[core // 4] += np.asarray(r.results[core]["out"], dtype=np.float32)
    full += np.asarray(b_proj, np.float32)[None, None, :]
    return full, r.exec_time_ns


def kernel(**inputs):
    x = np.asarray(inputs["x"], np.float32)
    w_qkv = np.asarray(inputs["w_qkv"], np.float32)
    w_proj = np.asarray(inputs["w_proj"], np.float32)
    b_proj = np.asarray(inputs["b_proj"], np.float32)
    out, _ = run_hw(x, w_qkv, w_proj, b_proj, trace=False)
    return out



# revision 42
# speedup vs baseline: 1.0138x; 1.0138x over previous
"""Fused multi-head attention (B=2, N=2048, C=1024, H=16) on 8 TRN2 NeuronCores.

Sharding: core = (b, g) with b = batch (2) and g = head-group of 4 heads (4).
Each core computes, for its batch and 4 heads:
    qkv slice -> per-head softmax attention -> out-proj partial (row-parallel).
Host sums the 4 per-head-group proj partials per batch and adds b_proj.

Device algorithm (per core), matmuls in bf16 (default) or float32r (TF32):
  phase 1: qkT = (x @ Wqk)^T   [q/k feats on partitions, 2048 tokens]
           v   = x @ Wv        [2048 tokens, 4*64] (+ ones column per head)
  phase 2: per (head pair, 512-row chunk):
           S^T tiles = matmul(lhsT=kTp_h, rhs=q-chunk)  [128 keys, 512 rows]
             kTp is K=128 zero-padded per head (even head rows 0:64, odd
             64:128) so full-array matmuls select one head's contraction
           expST = exp(S^T/8)  (ScalarE, PSUM->SBUF, pairs of key chunks)
           outT[65, rows] += [v_h|1]^T-matmul expST  (K=128 keys)
             row 64 = softmax denominator (ones column trick)
           outT[0:64] *= 1/denominator  (DVE recip, GpSimd bcast, DVE mult)
  phase 3: partial = out^T-matmul Wp -> bf16 -> DMA out

Schedule: a minimal prologue (q+k heads01 nt0, v keys 0:256) starts attention
block (0,0) at ~7us; ALL remaining qkv work rides the fill queue inside the
blocks' PE slack, paced per kc2 step so kTp/v chunks land just before their
consumers (first exp at ~12us vs ~48us for a separate qkv phase).  Inputs are
token-major and split across the sync/gpsimd/scalar DMA queues (the ~650ns
dma_start queue cost gates the prologue); proj output stages two 512-col
halves into one [128,1024] bf16 tile so each token chunk is a single 2KB-row
DMA; proj fill-pops spread 1/step through the exp-paced blocks.  The final
block normalizes outT per 128-token chunk and launches that chunk's proj+DMA
immediately, with the denominator staged via the then-idle ScalarE so the PE
clock stays warm through the tail.

Totals per core: PE ~191us busy (the binding engine; streaming floor 164us),
ScalarE exp ~150us (128 ACTs of [128,1024]; PSUM's 8x2KB banks cannot fit
wider double-buffered ST tiles, bf16 matmul PSUM output is TRN3-only, and
DVE-staged SBUF exp regressed ~50us -- serialization beat the ACT savings),
DVE ~92us.  HW exec ~224us in the machine's fast state (baseline 253.7us =
separate-phase schedule + 4us-per-head iterative-divide reciprocal).
Note: the box drifts between "fast" and ~20% slower power states on minute
timescales; compare variants only via interleaved runs in one process
(bench.py).
"""

import os
from contextlib import ExitStack

import numpy as np

import concourse.bass as bass
import concourse.mybir as mybir
import concourse.tile as tile
from concourse import bacc
from concourse.bass_utils import run_bass_kernel_spmd

B, N, C = 2, 2048, 1024
HC = 4  # heads per core
D = 64
NCORES = 8
KC = C // 128  # 8 contraction chunks for phase 1
SCALE = D**-0.5  # 0.125

# "f32r" (fp32 data, full-rate PE mode), "bf16", or "f32" (4x slower PE)
MM_DT = os.environ.get("ATTN_MM_DT", "bf16")
ST_TILE_POS = os.environ.get("ATTN_ST_TILE_POS", "1") == "1"
ACT_COPY = os.environ.get("ATTN_ACT_COPY", "0") == "1"


def _np_in_dtype():
    if MM_DT == "bf16":
        import ml_dtypes

        return np.dtype(ml_dtypes.bfloat16)
    return np.dtype(np.float32)


def _prep(a):
    """Cast to the device input dtype; for f32r, pre-round to TF32 (RTNE)."""
    a = np.ascontiguousarray(a)
    if MM_DT != "f32r":
        return a.astype(_np_in_dtype())
    u = a.astype(np.float32).view(np.uint32)
    u = (u + 0x0FFF + ((u >> 13) & 1)) & np.uint32(0xFFFFE000)
    return u.view(np.float32)



def _copy(eng, out, in_):
    if hasattr(eng, "tensor_copy"):
        eng.tensor_copy(out, in_)
    else:
        eng.copy(out, in_)

def build_nc():
    f32 = mybir.dt.float32
    in_dt = {
        "bf16": mybir.dt.bfloat16,
        "f32r": mybir.dt.float32r,
        "f32": mybir.dt.float32,
    }[MM_DT]
    mm = lambda ap: ap  # noqa: E731

    out_dt = mybir.dt.bfloat16 if MM_DT == "bf16" else f32

    nc = bacc.Bacc("TRN2", target_bir_lowering=False, debug=False, num_devices=NCORES)
    xT_d = nc.dram_tensor("xT", [C, N], in_dt, kind="ExternalInput").ap()
    wqk_d = nc.dram_tensor("wqk", [C, 2 * HC * D], in_dt, kind="ExternalInput").ap()
    wv_d = nc.dram_tensor("wv", [C, HC * D], in_dt, kind="ExternalInput").ap()
    wp_d = nc.dram_tensor("wp", [HC * D, C], in_dt, kind="ExternalInput").ap()
    # bf16 proj partials: halves the output DMA (the tail's critical path);
    # the host accumulates the 4 partials per batch in f32.
    out_d = nc.dram_tensor("out", [N, C], out_dt, kind="ExternalOutput").ap()

    with tile.TileContext(nc) as tc:
        with (
            tc.tile_pool(name="const", bufs=1) as const,
            tc.tile_pool(name="ex", bufs=8) as expool,
            tc.tile_pool(name="den", bufs=6) as dpool,
            tc.tile_pool(name="stage", bufs=4) as stage,
            tc.tile_pool(name="stps", bufs=2, space="PSUM") as stps,
            tc.tile_pool(name="pvps", bufs=4, space="PSUM") as pvps,
        ):
            # persistent tiles
            # qkT chunks: 0 = q heads 0,1; 1 = q heads 2,3
            #   (head even -> partitions 0:64, odd -> 64:128)
            # kTp: per-head zero-padded K=128 stationary operand: head even
            #   has kT in rows 0:64 / zeros in 64:128, head odd the reverse,
            #   so a full-128-row matmul against the stacked q chunk
            #   contracts only the matching head's 64 features.
            qkT_sb = const.tile([128, 2, N], in_dt, tag="qkT")
            kTp_sb = const.tile([128, HC, N], in_dt, tag="kTp")
            v_sb = const.tile([128, 16, HC, D + 1], in_dt, tag="v")
            wp_sb = const.tile([128, 2, C], in_dt, tag="wp")
            outT_sb = const.tile([128, 2, N], in_dt, tag="outT")
            xT_sb = const.tile([128, KC, N], in_dt, tag="xT")
            wqk_sb = const.tile([128, KC, 2 * HC * D], in_dt, tag="wqk")
            wv_sb = const.tile([128, KC, HC * D], in_dt, tag="wv")

            # ---- DMAs, token-major for xT: wqk/xT(nt=0) interleaved so the
            # prologue qk chains start after ~0.5MB; attention block (0,0)
            # can then begin at ~6us instead of ~40us.
            # Inputs split across the Sync and GpSimd HW DMA queues so two
            # engines drain in parallel; the prologue-critical wqk+xT(nt=0)
            # chunks alternate queues kc-wise to land first.
            # The ~650ns/dma_start queue cost gates the prologue, so the 16
            # critical transfers (wqk + xT nt0) go 3-way across sync, gpsimd
            # AND the (otherwise idle at startup) scalar queue.
            qs = [nc.sync, nc.gpsimd, nc.scalar]

            def dma_xt(nt, q):
                for kc in range(KC):
                    q.dma_start(
                        xT_sb[:, kc, nt * 512 : (nt + 1) * 512],
                        xT_d[kc * 128 : (kc + 1) * 128, nt * 512 : (nt + 1) * 512],
                    )

            for kc in range(KC):
                qs[(2 * kc) % 3].dma_start(
                    wqk_sb[:, kc, :], wqk_d[kc * 128 : (kc + 1) * 128, :]
                )
                qs[(2 * kc + 1) % 3].dma_start(
                    xT_sb[:, kc, 0:512], xT_d[kc * 128 : (kc + 1) * 128, 0:512]
                )
            for kc in range(KC):
                qs[kc % 2].dma_start(wv_sb[:, kc, :], wv_d[kc * 128 : (kc + 1) * 128, :])
            dma_xt(1, nc.gpsimd)
            dma_xt(2, nc.sync)
            dma_xt(3, nc.gpsimd)
            for c2 in range(2):
                nc.sync.dma_start(wp_sb[:, c2, :], wp_d[c2 * 128 : (c2 + 1) * 128, :])

            # ---- one-time fills (run during the DMA wait) ----
            zsrc = const.tile([64, 512], f32, tag="zsrc")
            nc.vector.memset(zsrc[:], 0.0)
            for h in range(HC):
                zb = 64 if h % 2 == 0 else 0
                for nt in range(4):
                    nc.vector.tensor_copy(
                        kTp_sb[zb : zb + 64, h, nt * 512 : (nt + 1) * 512], zsrc[:]
                    )
            ones_f32 = const.tile([128, 16, HC, 1], f32, tag="ones")
            nc.vector.memset(ones_f32[:], 1.0)
            nc.vector.tensor_copy(v_sb[:, :, :, D : D + 1], ones_f32[:])

            # ---- emission helpers ----
            def qk_chunk(mf, nt):
                """One psum of (x @ Wqk)^T: feat chunk mf, token chunk nt.
                wqk feat chunks: 0 = q heads 0,1; 1 = q heads 2,3;
                2 = k heads 0,1; 3 = k heads 2,3."""
                ps = pvps.tile([128, 512], f32, tag="pv", name="pv")
                for kc in range(KC):
                    nc.tensor.matmul(
                        ps,
                        mm(wqk_sb[:, kc, mf * 128 : (mf + 1) * 128]),
                        mm(xT_sb[:, kc, nt * 512 : (nt + 1) * 512]),
                        start=(kc == 0),
                        stop=(kc == KC - 1),
                    )
                nts = slice(nt * 512, (nt + 1) * 512)
                if mf % 2 == 0:
                    mq = mf // 2
                    if mf < 2:
                        nc.vector.tensor_copy(qkT_sb[:, mq, nts], ps)
                    else:
                        pass
                if mf < 2:
                    if mf == 1:
                        nc.vector.tensor_copy(qkT_sb[:, 1, nts], ps)
                else:
                    h0, h1 = 2 * (mf - 2), 2 * (mf - 2) + 1
                    nc.vector.tensor_copy(kTp_sb[0:64, h0, nts], ps[0:64, :])
                    nc.vector.tensor_copy(kTp_sb[64:128, h1, nts], ps[64:128, :])

            def v_chunk(t):
                """One psum of v = x @ Wv for token(=key) chunk t, all heads."""
                ps = pvps.tile([128, 512], f32, tag="pv", name="pv")[:, : HC * D]
                for kc in range(KC):
                    nc.tensor.matmul(
                        ps,
                        mm(xT_sb[:, kc, t * 128 : (t + 1) * 128]),
                        mm(wv_sb[:, kc, :]),
                        start=(kc == 0),
                        stop=(kc == KC - 1),
                    )
                nc.vector.tensor_copy(
                    v_sb[:, t, :, 0:D], ps.rearrange("p (h d) -> p h d", h=HC)
                )

            sg2_of = {}

            def proj_chunk(t, nf):
                """partial[t*128:(t+1)*128, nf*512:(nf+1)*512] = out @ Wp.
                Both nf halves stage into one [128,1024] tile; the DMA (2KB
                rows, half the packets) fires once per token chunk."""
                ps = pvps.tile([128, 512], f32, tag="pv", name="pv")
                for c2 in range(2):
                    nc.tensor.matmul(
                        ps,
                        mm(outT_sb[:, c2, t * 128 : (t + 1) * 128]),
                        mm(wp_sb[:, c2, nf * 512 : (nf + 1) * 512]),
                        start=(c2 == 0),
                        stop=(c2 == 1),
                    )
                if nf == 0:
                    sg2_of[t] = stage.tile(
                        [128, 1024], out_dt, tag="sg2", name="sg2", bufs=2
                    )
                sg = sg2_of[t]
                nc.vector.tensor_copy(sg[:, nf * 512 : (nf + 1) * 512], ps)
                if nf == 1:
                    nc.sync.dma_start(out_d[t * 128 : (t + 1) * 128, :], sg)
                    del sg2_of[t]

            def proj_tail(t):
                """Both nf halves of token chunk t in one stps-pool psum
                (free after the last exp): fewer, wider tail ops + 2KB-row
                output DMA."""
                ps = stps.tile([128, 1024], f32, tag="st", name="st")
                for nf in range(2):
                    for c2 in range(2):
                        nc.tensor.matmul(
                            ps[:, nf * 512 : (nf + 1) * 512],
                            mm(outT_sb[:, c2, t * 128 : (t + 1) * 128]),
                            mm(wp_sb[:, c2, nf * 512 : (nf + 1) * 512]),
                            start=(c2 == 0),
                            stop=(c2 == 1),
                        )
                sg = stage.tile([128, 1024], out_dt, tag="sg2", name="sg2", bufs=2)
                nc.vector.tensor_copy(sg, ps)
                # tail runs after the last ACT, so the scalar queue is free
                (nc.sync if t % 2 == 0 else nc.scalar).dma_start(
                    out_d[t * 128 : (t + 1) * 128, :], sg
                )

            # fill queue: work interleaved into attention blocks' PE slack
            fills = []

            def attention_block(hp, rc, sched, tail=False):
                """ST + exp + PV for head pair hp, 512-row chunk rc; pops
                sched[kc2] fill closures at the top of each kc2 step."""
                heads = (2 * hp, 2 * hp + 1)
                pv = {
                    h: pvps.tile([128, 512], f32, tag="pv", name="pv") for h in heads
                }
                for kc2 in range(8):
                    for _ in range(sched[kc2]):
                        if fills:
                            fills.pop(0)()
                    stp = {
                        h: stps.tile([128, 1024], f32, tag="st", name="st")
                        for h in heads
                    }
                    for j in range(2):
                        kc = 2 * kc2 + j
                        for h in heads:
                            nc.tensor.matmul(
                                stp[h][:, j * 512 : (j + 1) * 512],
                                mm(kTp_sb[:, h, kc * 128 : (kc + 1) * 128]),
                                mm(qkT_sb[:, hp, rc * 512 : (rc + 1) * 512]),
                                start=True,
                                stop=True,
                            )
                    for h in heads:
                        ex = expool.tile([128, 1024], in_dt, tag="ex", name="ex")
                        nc.scalar.activation(
                            ex, stp[h], mybir.ActivationFunctionType.Exp, scale=SCALE
                        )
                        for j in range(2):
                            kc = 2 * kc2 + j
                            nc.tensor.matmul(
                                pv[h][: D + 1, :],
                                mm(v_sb[:, kc, h, :]),
                                mm(ex[:, j * 512 : (j + 1) * 512]),
                                start=(kc == 0),
                                stop=(kc == 15),
                            )
                # both recips first: DVE stays busy while GpSimd runs the
                # first broadcast, finishing the chain ~1us sooner (keeps the
                # post-block stall under the 3.4us HAM re-throttle window).
                # approx-fast recip: ~51 ULP, ~5x faster than the iterative
                # divide -- the denominator only needs ~1e-2 relative.
                dens, rbcs = {}, {}
                for h in heads:
                    dens[h] = dpool.tile([1, 512], f32, tag="den", name="den")
                    if os.environ.get("ATTN_RECIP", "fast") == "fast":
                        dsrc = dpool.tile([1, 512], f32, tag="dsrc", name="dsrc")
                        # tail: ScalarE is idle after the last exp -- staging
                        # the denominator there keeps the PE-idle gap under
                        # the ~3.4us HAM re-throttle window, so the tail proj
                        # matmuls run at full clock.
                        if tail:
                            nc.scalar.copy(dsrc, pv[h][D : D + 1, :])
                        else:
                            nc.vector.tensor_copy(dsrc, pv[h][D : D + 1, :])
                        nc.vector.reciprocal_approx_fast(out=dens[h], in_=dsrc)
                    else:
                        nc.vector.reciprocal(dens[h], pv[h][D : D + 1, :])
                for h in heads:
                    rbcs[h] = dpool.tile([64, 512], f32, tag="rbc", name="rbc")
                    nc.gpsimd.partition_broadcast(rbcs[h], dens[h])
                if not tail:
                    for h in heads:
                        hb = (h % 2) * 64
                        nc.vector.tensor_tensor(
                            out=outT_sb[hb : hb + 64, hp, rc * 512 : (rc + 1) * 512],
                            in0=pv[h][0:D, :],
                            in1=rbcs[h][:],
                            op=mybir.AluOpType.mult,
                        )
                else:
                    # final block: normalize per 128-token chunk and launch
                    # that chunk's out-proj + DMA immediately, so the tail
                    # pipeline (mult -> proj MM -> cast -> DMA) overlaps
                    # instead of serializing after the whole block.
                    for tc4 in range(4):
                        ts = slice(rc * 512 + tc4 * 128, rc * 512 + tc4 * 128 + 128)
                        for h in heads:
                            hb = (h % 2) * 64
                            nc.vector.tensor_tensor(
                                out=outT_sb[hb : hb + 64, hp, ts],
                                in0=pv[h][0:D, tc4 * 128 : (tc4 + 1) * 128],
                                in1=rbcs[h][:, tc4 * 128 : (tc4 + 1) * 128],
                                op=mybir.AluOpType.mult,
                            )
                        proj_tail(4 * rc + tc4)

            # ---- schedule ----
            def queue_proj(rc):
                fills.extend(
                    [
                        lambda t=t, nf=nf: proj_chunk(t, nf)
                        for t in range(4 * rc, 4 * rc + 4)
                        for nf in range(2)
                    ]
                )

            # Minimal prologue: exactly the chunks block (0,0) step 0 needs
            # (q heads01 rc0, kTp keys 0:256, v keys 0:256). Everything else
            # rides the fill queue inside the attention blocks, so ScalarE
            # exp starts at ~7us instead of ~40us and the PE is never
            # DMA-starved for long (HAM stays warm).
            qk_chunk(0, 0)
            qk_chunk(2, 0)
            v_chunk(0)
            v_chunk(1)
            # Fill order: v(2k),v(2k+1) land just before their PV(k) consumer
            # in block (0,0); kTp chunk qk(2,nt) before the step contracting
            # those keys; q chunks qk(0,1..3) before blocks (0,1..3).
            fills.extend(
                [
                    lambda: v_chunk(2),
                    lambda: v_chunk(3),
                    lambda: qk_chunk(2, 1),
                    lambda: v_chunk(4),
                    lambda: v_chunk(5),
                    lambda: v_chunk(6),
                    lambda: v_chunk(7),
                    lambda: qk_chunk(2, 2),
                    lambda: v_chunk(8),
                    lambda: v_chunk(9),
                    lambda: v_chunk(10),
                    lambda: v_chunk(11),
                    lambda: qk_chunk(2, 3),
                    lambda: v_chunk(12),
                    lambda: v_chunk(13),
                    lambda: qk_chunk(0, 1),
                    lambda: v_chunk(14),
                    lambda: v_chunk(15),
                ]
            )
            attention_block(0, 0, sched=[0, 3, 2, 3, 2, 3, 2, 3])
            fills.append(lambda: qk_chunk(0, 2))
            fills.append(lambda: qk_chunk(0, 3))
            fills.extend([lambda nt=nt: qk_chunk(1, nt) for nt in range(4)])
            fills.extend([lambda nt=nt: qk_chunk(3, nt) for nt in range(4)])
            # Alternating hp order: each (1,rc) block's denominator chain gets
            # a full unrelated block of latency before proj(rc) pops consume
            # its outT, so the PE never stalls on the recip/bcast/mult chain.
            late = [0, 1, 1, 1, 1, 1, 1, 1]
            attention_block(0, 1, sched=[2, 2, 2, 1, 1, 1, 1, 1])
            attention_block(1, 0, sched=[0] * 8)
            queue_proj(0)
            attention_block(0, 2, sched=[0] * 8)
            attention_block(1, 1, sched=late)
            queue_proj(1)
            attention_block(0, 3, sched=[0, 0, 0, 1, 1, 0, 0, 0])
            attention_block(1, 2, sched=late)
            queue_proj(2)
            attention_block(1, 3, sched=[0, 0, 0, 2, 2, 3, 1, 1], tail=True)
            # drain any straggler fills (none expected)
            while fills:
                fills.pop(0)()
    nc.compile()
    return nc


def make_in_maps(x, w_qkv, w_proj):
    in_maps = []
    for core in range(NCORES):
        b, g = core // 4, core % 4
        qs = slice(g * 256, (g + 1) * 256)
        in_maps.append(
            {
                "xT": _prep(x[b].T),
                "wqk": _prep(
                    np.concatenate(
                        [w_qkv[:, qs], w_qkv[:, C + g * 256 : C + (g + 1) * 256]],
                        axis=1,
                    )
                ),
                "wv": _prep(w_qkv[:, 2 * C + g * 256 : 2 * C + (g + 1) * 256]),
                "wp": _prep(w_proj[qs, :]),
            }
        )
    return in_maps


def run_hw(x, w_qkv, w_proj, b_proj, trace=False, tmpdir=None):
    """Returns (full output [2, 2048, 1024] f32, exec_time_ns or None)."""
    in_maps = make_in_maps(x, w_qkv, w_proj)
    nc = build_nc()
    r = run_bass_kernel_spmd(
        nc, in_maps, core_ids=list(range(NCORES)), trace=trace, tmpdir=tmpdir
    )
    full = np.zeros((B, N, C), np.float32)
    for core in range(NCORES):
        full[core // 4] += np.asarray(r.results[core]["out"], dtype=np.float32)
    full += np.asarray(b_proj, np.float32)[None, None, :]
    return full, r.exec_time_ns


def kernel(**inputs):
    x = np.asarray(inputs["x"], np.float32)
    w_qkv = np.asarray(inputs["w_qkv"], np.float32)
    w_proj = np.asarray(inputs["w_proj"], np.float32)
    b_proj = np.asarray(inputs["b_proj"], np.float32)
    out, _ = run_hw(x, w_qkv, w_proj, b_proj, trace=False)
    return out



# revision 43
# speedup vs baseline: 1.1842x; 1.1681x over previous
"""Fused multi-head attention (B=2, N=2048, C=1024, H=16) on 8 TRN2 NeuronCores.

Sharding: core = (b, g) with b = batch (2) and g = head-group of 4 heads (4).
Each core computes, for its batch and 4 heads:
    qkv slice -> per-head softmax attention -> out-proj partial (row-parallel).
Host sums the 4 per-head-group proj partials per batch and adds b_proj.

Device algorithm (per core), matmuls in bf16 (default) or float32r (TF32):
  phase 1: qkT = (x @ Wqk)^T   [q/k feats on partitions, 2048 tokens]
           v   = x @ Wv        [2048 tokens, 4*64] (+ ones column per head)
  phase 2: per (head pair, 512-row chunk):
           S^T tiles = matmul(lhsT=kTp_h, rhs=q-chunk)  [128 keys, 512 rows]
             kTp is K=128 zero-padded per head (even head rows 0:64, odd
             64:128) so full-array matmuls select one head's contraction
           expST = exp(S^T/8)  (ScalarE, PSUM->SBUF, pairs of key chunks)
           outT[65, rows] += [v_h|1]^T-matmul expST  (K=128 keys)
             row 64 = softmax denominator (ones column trick)
           outT[0:64] *= 1/denominator  (DVE recip, GpSimd bcast, DVE mult)
  phase 3: partial = out^T-matmul Wp -> bf16 -> DMA out

Schedule: a minimal prologue (q+k heads01 nt0, v keys 0:256) starts attention
block (0,0) at ~7us; ALL remaining qkv work rides the fill queue inside the
blocks' PE slack, paced per kc2 step so kTp/v chunks land just before their
consumers (first exp at ~12us vs ~48us for a separate qkv phase).  Inputs are
token-major and split across the sync/gpsimd/scalar DMA queues (the ~650ns
dma_start queue cost gates the prologue); proj output stages two 512-col
halves into one [128,1024] bf16 tile so each token chunk is a single 2KB-row
DMA; proj fill-pops spread 1/step through the exp-paced blocks.  The final
block normalizes outT per 128-token chunk and launches that chunk's proj+DMA
immediately, with the denominator staged via the then-idle ScalarE so the PE
clock stays warm through the tail.

Totals per core: PE ~191us busy (the binding engine; streaming floor 164us),
ScalarE exp ~150us (128 ACTs of [128,1024]; PSUM's 8x2KB banks cannot fit
wider double-buffered ST tiles, bf16 matmul PSUM output is TRN3-only, and
DVE-staged SBUF exp regressed ~50us -- serialization beat the ACT savings),
DVE ~92us.  HW exec ~224us in the machine's fast state (baseline 253.7us =
separate-phase schedule + 4us-per-head iterative-divide reciprocal).
Note: the box drifts between "fast" and ~20% slower power states on minute
timescales; compare variants only via interleaved runs in one process
(bench.py).
"""

import os
from contextlib import ExitStack

import numpy as np

import concourse.bass as bass
import concourse.mybir as mybir
import concourse.tile as tile
from concourse import bacc
from concourse.bass_utils import run_bass_kernel_spmd

B, N, C = 2, 2048, 1024
HC = 4  # heads per core
D = 64
NCORES = 8
KC = C // 128  # 8 contraction chunks for phase 1
SCALE = D**-0.5  # 0.125

# "f32r" (fp32 data, full-rate PE mode), "bf16", or "f32" (4x slower PE)
MM_DT = os.environ.get("ATTN_MM_DT", "bf16")
ST_TILE_POS = os.environ.get("ATTN_ST_TILE_POS", "1") == "1"
ACT_COPY = os.environ.get("ATTN_ACT_COPY", "0") == "1"


def _np_in_dtype():
    if MM_DT == "bf16":
        import ml_dtypes

        return np.dtype(ml_dtypes.bfloat16)
    return np.dtype(np.float32)


def _prep(a):
    """Cast to the device input dtype; for f32r, pre-round to TF32 (RTNE)."""
    a = np.ascontiguousarray(a)
    if MM_DT != "f32r":
        return a.astype(_np_in_dtype())
    u = a.astype(np.float32).view(np.uint32)
    u = (u + 0x0FFF + ((u >> 13) & 1)) & np.uint32(0xFFFFE000)
    return u.view(np.float32)



def _copy(eng, out, in_):
    if hasattr(eng, "tensor_copy"):
        eng.tensor_copy(out, in_)
    else:
        eng.copy(out, in_)

def build_nc():
    f32 = mybir.dt.float32
    in_dt = {
        "bf16": mybir.dt.bfloat16,
        "f32r": mybir.dt.float32r,
        "f32": mybir.dt.float32,
    }[MM_DT]
    mm = lambda ap: ap  # noqa: E731

    out_dt = mybir.dt.bfloat16 if MM_DT == "bf16" else f32

    nc = bacc.Bacc("TRN2", target_bir_lowering=False, debug=False, num_devices=NCORES)
    xT_d = nc.dram_tensor("xT", [C, N], in_dt, kind="ExternalInput").ap()
    wqk_d = nc.dram_tensor("wqk", [C, 2 * HC * D], in_dt, kind="ExternalInput").ap()
    wv_d = nc.dram_tensor("wv", [C, HC * D], in_dt, kind="ExternalInput").ap()
    wp_d = nc.dram_tensor("wp", [HC * D, C], in_dt, kind="ExternalInput").ap()
    # bf16 proj partials: halves the output DMA (the tail's critical path);
    # the host accumulates the 4 partials per batch in f32.
    out_d = nc.dram_tensor("out", [N, C], out_dt, kind="ExternalOutput").ap()

    with tile.TileContext(nc) as tc:
        with (
            tc.tile_pool(name="const", bufs=1) as const,
            tc.tile_pool(name="ex", bufs=8) as expool,
            tc.tile_pool(name="den", bufs=6) as dpool,
            tc.tile_pool(name="stage", bufs=4) as stage,
            tc.tile_pool(name="stps", bufs=2, space="PSUM") as stps,
            tc.tile_pool(name="pvps", bufs=4, space="PSUM") as pvps,
        ):
            # persistent tiles
            # qkT chunks: 0 = q heads 0,1; 1 = q heads 2,3
            #   (head even -> partitions 0:64, odd -> 64:128)
            # kTp: per-head zero-padded K=128 stationary operand: head even
            #   has kT in rows 0:64 / zeros in 64:128, head odd the reverse,
            #   so a full-128-row matmul against the stacked q chunk
            #   contracts only the matching head's 64 features.
            qkT_sb = const.tile([128, 2, N], in_dt, tag="qkT")
            kTp_sb = const.tile([128, HC, N], in_dt, tag="kTp")
            v_sb = const.tile([128, 16, HC, D + 1], in_dt, tag="v")
            wp_sb = const.tile([128, 2, C], in_dt, tag="wp")
            outT_sb = const.tile([128, 2, N], in_dt, tag="outT")
            xT_sb = const.tile([128, KC, N], in_dt, tag="xT")
            wqk_sb = const.tile([128, KC, 2 * HC * D], in_dt, tag="wqk")
            wv_sb = const.tile([128, KC, HC * D], in_dt, tag="wv")

            # ---- DMAs, token-major for xT: wqk/xT(nt=0) interleaved so the
            # prologue qk chains start after ~0.5MB; attention block (0,0)
            # can then begin at ~6us instead of ~40us.
            # Inputs split across the Sync and GpSimd HW DMA queues so two
            # engines drain in parallel; the prologue-critical wqk+xT(nt=0)
            # chunks alternate queues kc-wise to land first.
            # The ~650ns/dma_start queue cost gates the prologue, so the 16
            # critical transfers (wqk + xT nt0) go 3-way across sync, gpsimd
            # AND the (otherwise idle at startup) scalar queue.
            qs = [nc.sync, nc.gpsimd, nc.scalar]

            def dma_xt(nt, q):
                for kc in range(KC):
                    q.dma_start(
                        xT_sb[:, kc, nt * 512 : (nt + 1) * 512],
                        xT_d[kc * 128 : (kc + 1) * 128, nt * 512 : (nt + 1) * 512],
                    )

            for kc in range(KC):
                qs[(2 * kc) % 3].dma_start(
                    wqk_sb[:, kc, :], wqk_d[kc * 128 : (kc + 1) * 128, :]
                )
                qs[(2 * kc + 1) % 3].dma_start(
                    xT_sb[:, kc, 0:512], xT_d[kc * 128 : (kc + 1) * 128, 0:512]
                )
            for kc in range(KC):
                qs[kc % 2].dma_start(wv_sb[:, kc, :], wv_d[kc * 128 : (kc + 1) * 128, :])
            dma_xt(1, nc.gpsimd)
            dma_xt(2, nc.sync)
            dma_xt(3, nc.gpsimd)
            for c2 in range(2):
                nc.sync.dma_start(wp_sb[:, c2, :], wp_d[c2 * 128 : (c2 + 1) * 128, :])

            # ---- one-time fills (run during the DMA wait) ----
            zsrc = const.tile([64, 512], f32, tag="zsrc")
            nc.vector.memset(zsrc[:], 0.0)
            for h in range(HC):
                zb = 64 if h % 2 == 0 else 0
                for nt in range(4):
                    nc.vector.tensor_copy(
                        kTp_sb[zb : zb + 64, h, nt * 512 : (nt + 1) * 512], zsrc[:]
                    )
            ones_f32 = const.tile([128, 16, HC, 1], f32, tag="ones")
            nc.vector.memset(ones_f32[:], 1.0)
            nc.vector.tensor_copy(v_sb[:, :, :, D : D + 1], ones_f32[:])

            # ---- emission helpers ----
            def qk_chunk(mf, nt):
                """One psum of (x @ Wqk)^T: feat chunk mf, token chunk nt.
                wqk feat chunks: 0 = q heads 0,1; 1 = q heads 2,3;
                2 = k heads 0,1; 3 = k heads 2,3."""
                ps = pvps.tile([128, 512], f32, tag="pv", name="pv")
                for kc in range(KC):
                    nc.tensor.matmul(
                        ps,
                        mm(wqk_sb[:, kc, mf * 128 : (mf + 1) * 128]),
                        mm(xT_sb[:, kc, nt * 512 : (nt + 1) * 512]),
                        start=(kc == 0),
                        stop=(kc == KC - 1),
                    )
                nts = slice(nt * 512, (nt + 1) * 512)
                if mf % 2 == 0:
                    mq = mf // 2
                    if mf < 2:
                        nc.vector.tensor_copy(qkT_sb[:, mq, nts], ps)
                    else:
                        pass
                if mf < 2:
                    if mf == 1:
                        nc.vector.tensor_copy(qkT_sb[:, 1, nts], ps)
                else:
                    h0, h1 = 2 * (mf - 2), 2 * (mf - 2) + 1
                    nc.vector.tensor_copy(kTp_sb[0:64, h0, nts], ps[0:64, :])
                    nc.vector.tensor_copy(kTp_sb[64:128, h1, nts], ps[64:128, :])

            def v_chunk(t):
                """One psum of v = x @ Wv for token(=key) chunk t, all heads."""
                ps = pvps.tile([128, 512], f32, tag="pv", name="pv")[:, : HC * D]
                for kc in range(KC):
                    nc.tensor.matmul(
                        ps,
                        mm(xT_sb[:, kc, t * 128 : (t + 1) * 128]),
                        mm(wv_sb[:, kc, :]),
                        start=(kc == 0),
                        stop=(kc == KC - 1),
                    )
                nc.vector.tensor_copy(
                    v_sb[:, t, :, 0:D], ps.rearrange("p (h d) -> p h d", h=HC)
                )

            sg2_of = {}

            def proj_chunk(t, nf):
                """partial[t*128:(t+1)*128, nf*512:(nf+1)*512] = out @ Wp.
                Both nf halves stage into one [128,1024] tile; the DMA (2KB
                rows, half the packets) fires once per token chunk."""
                ps = pvps.tile([128, 512], f32, tag="pv", name="pv")
                for c2 in range(2):
                    nc.tensor.matmul(
                        ps,
                        mm(outT_sb[:, c2, t * 128 : (t + 1) * 128]),
                        mm(wp_sb[:, c2, nf * 512 : (nf + 1) * 512]),
                        start=(c2 == 0),
                        stop=(c2 == 1),
                    )
                if nf == 0:
                    sg2_of[t] = stage.tile(
                        [128, 1024], out_dt, tag="sg2", name="sg2", bufs=2
                    )
                sg = sg2_of[t]
                nc.vector.tensor_copy(sg[:, nf * 512 : (nf + 1) * 512], ps)
                if nf == 1:
                    nc.sync.dma_start(out_d[t * 128 : (t + 1) * 128, :], sg)
                    del sg2_of[t]

            def proj_tail(t):
                """Both nf halves of token chunk t in one stps-pool psum
                (free after the last exp): fewer, wider tail ops + 2KB-row
                output DMA."""
                ps = stps.tile([128, 1024], f32, tag="st", name="st")
                for nf in range(2):
                    for c2 in range(2):
                        nc.tensor.matmul(
                            ps[:, nf * 512 : (nf + 1) * 512],
                            mm(outT_sb[:, c2, t * 128 : (t + 1) * 128]),
                            mm(wp_sb[:, c2, nf * 512 : (nf + 1) * 512]),
                            start=(c2 == 0),
                            stop=(c2 == 1),
                        )
                sg = stage.tile([128, 1024], out_dt, tag="sg2", name="sg2", bufs=2)
                nc.vector.tensor_copy(sg, ps)
                # tail runs after the last ACT, so the scalar queue is free
                (nc.sync if t % 2 == 0 else nc.scalar).dma_start(
                    out_d[t * 128 : (t + 1) * 128, :], sg
                )

            # fill queue: work interleaved into attention blocks' PE slack
            fills = []

            def attention_block(hp, rc, sched, tail=False):
                """ST + exp + PV for head pair hp, 512-row chunk rc; pops
                sched[kc2] fill closures at the top of each kc2 step."""
                heads = (2 * hp, 2 * hp + 1)
                pv = {
                    h: pvps.tile([128, 512], f32, tag="pv", name="pv") for h in heads
                }
                for kc2 in range(8):
                    for _ in range(sched[kc2]):
                        if fills:
                            fills.pop(0)()
                    stp = {
                        h: stps.tile([128, 1024], f32, tag="st", name="st")
                        for h in heads
                    }
                    for j in range(2):
                        kc = 2 * kc2 + j
                        for h in heads:
                            nc.tensor.matmul(
                                stp[h][:, j * 512 : (j + 1) * 512],
                                mm(kTp_sb[:, h, kc * 128 : (kc + 1) * 128]),
                                mm(qkT_sb[:, hp, rc * 512 : (rc + 1) * 512]),
                                start=True,
                                stop=True,
                            )
                    for h in heads:
                        ex = expool.tile([128, 1024], in_dt, tag="ex", name="ex")
                        nc.scalar.activation(
                            ex, stp[h], mybir.ActivationFunctionType.Exp, scale=SCALE
                        )
                        for j in range(2):
                            kc = 2 * kc2 + j
                            nc.tensor.matmul(
                                pv[h][: D + 1, :],
                                mm(v_sb[:, kc, h, :]),
                                mm(ex[:, j * 512 : (j + 1) * 512]),
                                start=(kc == 0),
                                stop=(kc == 15),
                            )
                if tail:
                    # the tail denominator chain leaves the PE idle ~3.7us --
                    # just over the HAM re-throttle window, which would halve
                    # the clock for the final proj matmuls. Dependency-free
                    # filler matmuls (f32 on zsrc, never consumed) bridge it.
                    wmt = stps.tile([128, 1024], f32, tag="st", name="wmt")
                    for _ in range(8):
                        nc.tensor.matmul(
                            wmt[:, 0:256],
                            zsrc[:, 0:128],
                            zsrc[:, 0:256],
                            start=True,
                            stop=True,
                        )
                # both recips first: DVE stays busy while GpSimd runs the
                # first broadcast, finishing the chain ~1us sooner (keeps the
                # post-block stall under the 3.4us HAM re-throttle window).
                # approx-fast recip: ~51 ULP, ~5x faster than the iterative
                # divide -- the denominator only needs ~1e-2 relative.
                dens, rbcs = {}, {}
                for h in heads:
                    dens[h] = dpool.tile([1, 512], f32, tag="den", name="den")
                    if os.environ.get("ATTN_RECIP", "fast") == "fast":
                        dsrc = dpool.tile([1, 512], f32, tag="dsrc", name="dsrc")
                        # tail: ScalarE is idle after the last exp -- staging
                        # the denominator there keeps the PE-idle gap under
                        # the ~3.4us HAM re-throttle window, so the tail proj
                        # matmuls run at full clock.
                        if tail:
                            nc.scalar.copy(dsrc, pv[h][D : D + 1, :])
                        else:
                            nc.vector.tensor_copy(dsrc, pv[h][D : D + 1, :])
                        nc.vector.reciprocal_approx_fast(out=dens[h], in_=dsrc)
                    else:
                        nc.vector.reciprocal(dens[h], pv[h][D : D + 1, :])
                for h in heads:
                    rbcs[h] = dpool.tile([64, 512], f32, tag="rbc", name="rbc")
                    nc.gpsimd.partition_broadcast(rbcs[h], dens[h])
                if not tail:
                    for h in heads:
                        hb = (h % 2) * 64
                        nc.vector.tensor_tensor(
                            out=outT_sb[hb : hb + 64, hp, rc * 512 : (rc + 1) * 512],
                            in0=pv[h][0:D, :],
                            in1=rbcs[h][:],
                            op=mybir.AluOpType.mult,
                        )
                else:
                    # final block: normalize per 128-token chunk and launch
                    # that chunk's out-proj + DMA immediately, so the tail
                    # pipeline (mult -> proj MM -> cast -> DMA) overlaps
                    # instead of serializing after the whole block.
                    for tc4 in range(4):
                        ts = slice(rc * 512 + tc4 * 128, rc * 512 + tc4 * 128 + 128)
                        for h in heads:
                            hb = (h % 2) * 64
                            nc.vector.tensor_tensor(
                                out=outT_sb[hb : hb + 64, hp, ts],
                                in0=pv[h][0:D, tc4 * 128 : (tc4 + 1) * 128],
                                in1=rbcs[h][:, tc4 * 128 : (tc4 + 1) * 128],
                                op=mybir.AluOpType.mult,
                            )
                        proj_tail(4 * rc + tc4)

            # ---- schedule ----
            def queue_proj(rc):
                fills.extend(
                    [
                        lambda t=t, nf=nf: proj_chunk(t, nf)
                        for t in range(4 * rc, 4 * rc + 4)
                        for nf in range(2)
                    ]
                )

            # Minimal prologue: exactly the chunks block (0,0) step 0 needs
            # (q heads01 rc0, kTp keys 0:256, v keys 0:256). Everything else
            # rides the fill queue inside the attention blocks, so ScalarE
            # exp starts at ~7us instead of ~40us and the PE is never
            # DMA-starved for long (HAM stays warm).
            qk_chunk(0, 0)
            qk_chunk(2, 0)
            v_chunk(0)
            v_chunk(1)
            # Fill order: v(2k),v(2k+1) land just before their PV(k) consumer
            # in block (0,0); kTp chunk qk(2,nt) before the step contracting
            # those keys; q chunks qk(0,1..3) before blocks (0,1..3).
            fills.extend(
                [
                    lambda: v_chunk(2),
                    lambda: v_chunk(3),
                    lambda: qk_chunk(2, 1),
                    lambda: v_chunk(4),
                    lambda: v_chunk(5),
                    lambda: v_chunk(6),
                    lambda: v_chunk(7),
                    lambda: qk_chunk(2, 2),
                    lambda: v_chunk(8),
                    lambda: v_chunk(9),
                    lambda: v_chunk(10),
                    lambda: v_chunk(11),
                    lambda: qk_chunk(2, 3),
                    lambda: v_chunk(12),
                    lambda: v_chunk(13),
                    lambda: qk_chunk(0, 1),
                    lambda: v_chunk(14),
                    lambda: v_chunk(15),
                ]
            )
            attention_block(0, 0, sched=[0, 3, 2, 3, 2, 3, 2, 3])
            fills.append(lambda: qk_chunk(0, 2))
            fills.append(lambda: qk_chunk(0, 3))
            fills.extend([lambda nt=nt: qk_chunk(1, nt) for nt in range(4)])
            fills.extend([lambda nt=nt: qk_chunk(3, nt) for nt in range(4)])
            # Alternating hp order: each (1,rc) block's denominator chain gets
            # a full unrelated block of latency before proj(rc) pops consume
            # its outT, so the PE never stalls on the recip/bcast/mult chain.
            late = [0, 1, 1, 1, 1, 1, 1, 1]
            attention_block(0, 1, sched=[2, 2, 2, 1, 1, 1, 1, 1])
            attention_block(1, 0, sched=[0] * 8)
            queue_proj(0)
            attention_block(0, 2, sched=[0] * 8)
            attention_block(1, 1, sched=late)
            queue_proj(1)
            attention_block(0, 3, sched=[0, 0, 0, 1, 1, 0, 0, 0])
            attention_block(1, 2, sched=late)
            queue_proj(2)
            attention_block(1, 3, sched=[0, 0, 0, 2, 2, 3, 1, 1], tail=True)
            # drain any straggler fills (none expected)
            while fills:
                fills.pop(0)()
    nc.compile()
    return nc


def make_in_maps(x, w_qkv, w_proj):
    in_maps = []
    for core in range(NCORES):
        b, g = core // 4, core % 4
        qs = slice(g * 256, (g + 1) * 256)
        in_maps.append(
            {
                "xT": _prep(x[b].T),
                "wqk": _prep(
                    np.concatenate(
                        [w_qkv[:, qs], w_qkv[:, C + g * 256 : C + (g + 1) * 256]],
                        axis=1,
                    )
                ),
                "wv": _prep(w_qkv[:, 2 * C + g * 256 : 2 * C + (g + 1) * 256]),
                "wp": _prep(w_proj[qs, :]),
            }
        )
    return in_maps


def run_hw(x, w_qkv, w_proj, b_proj, trace=False, tmpdir=None):
    """Returns (full output [2, 2048, 1024] f32, exec_time_ns or None)."""
    in_maps = make_in_maps(x, w_qkv, w_proj)
    nc = build_nc()
    r = run_bass_kernel_spmd(
        nc, in_maps, core_ids=list(range(NCORES)), trace=trace, tmpdir=tmpdir
    )
    full = np.zeros((B, N, C), np.float32)
    for core in range(NCORES):
        full[core // 4] += np.asarray(r.results[core]["out"], dtype=np.float32)
    full += np.asarray(b_proj, np.float32)[None, None, :]
    return full, r.exec_time_ns


def kernel(**inputs):
    x = np.asarray(inputs["x"], np.float32)
    w_qkv = np.asarray(inputs["w_qkv"], np.float32)
    w_proj = np.asarray(inputs["w_proj"], np.float32)
    b_proj = np.asarray(inputs["b_proj"], np.float32)
    out, _ = run_hw(x, w_qkv, w_proj, b_proj, trace=False)
    return out



# revision 44
# speedup vs baseline: 1.1932x; 1.0076x over previous
"""Fused multi-head attention (B=2, N=2048, C=1024, H=16) on 8 TRN2 NeuronCores.

Sharding: core = (b, g) with b = batch (2) and g = head-group of 4 heads (4).
Each core computes, for its batch and 4 heads:
    qkv slice -> per-head softmax attention -> out-proj partial (row-parallel).
Host sums the 4 per-head-group proj partials per batch and adds b_proj.

Device algorithm (per core), matmuls in bf16 (default) or float32r (TF32):
  phase 1: qkT = (x @ Wqk)^T   [q/k feats on partitions, 2048 tokens]
           v   = x @ Wv        [2048 tokens, 4*64] (+ ones column per head)
  phase 2: per (head pair, 512-row chunk):
           S^T tiles = matmul(lhsT=kTp_h, rhs=q-chunk)  [128 keys, 512 rows]
             kTp is K=128 zero-padded per head (even head rows 0:64, odd
             64:128) so full-array matmuls select one head's contraction
           expST = exp(S^T/8)  (ScalarE, PSUM->SBUF, pairs of key chunks)
           outT[65, rows] += [v_h|1]^T-matmul expST  (K=128 keys)
             row 64 = softmax denominator (ones column trick)
           outT[0:64] *= 1/denominator  (DVE recip, GpSimd bcast, DVE mult)
  phase 3: partial = out^T-matmul Wp -> bf16 -> DMA out

Schedule: a minimal prologue (q+k heads01 nt0, v keys 0:256) starts attention
block (0,0) at ~7us; ALL remaining qkv work rides the fill queue inside the
blocks' PE slack, paced per kc2 step so kTp/v chunks land just before their
consumers (first exp at ~12us vs ~48us for a separate qkv phase).  Inputs are
token-major and split across the sync/gpsimd/scalar DMA queues (the ~650ns
dma_start queue cost gates the prologue); proj output stages two 512-col
halves into one [128,1024] bf16 tile so each token chunk is a single 2KB-row
DMA; proj fill-pops spread 1/step through the exp-paced blocks.  The final
block normalizes outT per 128-token chunk and launches that chunk's proj+DMA
immediately, with the denominator staged via the then-idle ScalarE so the PE
clock stays warm through the tail.

Totals per core: PE ~191us busy (the binding engine; streaming floor 164us),
ScalarE exp ~150us (128 ACTs of [128,1024]; PSUM's 8x2KB banks cannot fit
wider double-buffered ST tiles, bf16 matmul PSUM output is TRN3-only, and
DVE-staged SBUF exp regressed ~50us -- serialization beat the ACT savings),
DVE ~92us.  HW exec ~224us in the machine's fast state (baseline 253.7us =
separate-phase schedule + 4us-per-head iterative-divide reciprocal).
Note: the box drifts between "fast" and ~20% slower power states on minute
timescales; compare variants only via interleaved runs in one process
(bench.py).
"""

import os
from contextlib import ExitStack

import numpy as np

import concourse.bass as bass
import concourse.mybir as mybir
import concourse.tile as tile
from concourse import bacc
from concourse.bass_utils import run_bass_kernel_spmd

B, N, C = 2, 2048, 1024
HC = 4  # heads per core
D = 64
NCORES = 8
KC = C // 128  # 8 contraction chunks for phase 1
SCALE = D**-0.5  # 0.125

# "f32r" (fp32 data, full-rate PE mode), "bf16", or "f32" (4x slower PE)
MM_DT = os.environ.get("ATTN_MM_DT", "bf16")
ST_TILE_POS = os.environ.get("ATTN_ST_TILE_POS", "1") == "1"
ACT_COPY = os.environ.get("ATTN_ACT_COPY", "0") == "1"


def _np_in_dtype():
    if MM_DT == "bf16":
        import ml_dtypes

        return np.dtype(ml_dtypes.bfloat16)
    return np.dtype(np.float32)


def _prep(a):
    """Cast to the device input dtype; for f32r, pre-round to TF32 (RTNE)."""
    a = np.ascontiguousarray(a)
    if MM_DT != "f32r":
        return a.astype(_np_in_dtype())
    u = a.astype(np.float32).view(np.uint32)
    u = (u + 0x0FFF + ((u >> 13) & 1)) & np.uint32(0xFFFFE000)
    return u.view(np.float32)



def _copy(eng, out, in_):
    if hasattr(eng, "tensor_copy"):
        eng.tensor_copy(out, in_)
    else:
        eng.copy(out, in_)

def build_nc():
    f32 = mybir.dt.float32
    in_dt = {
        "bf16": mybir.dt.bfloat16,
        "f32r": mybir.dt.float32r,
        "f32": mybir.dt.float32,
    }[MM_DT]
    mm = lambda ap: ap  # noqa: E731

    out_dt = mybir.dt.bfloat16 if MM_DT == "bf16" else f32

    nc = bacc.Bacc("TRN2", target_bir_lowering=False, debug=False, num_devices=NCORES)
    xT_d = nc.dram_tensor("xT", [C, N], in_dt, kind="ExternalInput").ap()
    wqk_d = nc.dram_tensor("wqk", [C, 2 * HC * D], in_dt, kind="ExternalInput").ap()
    wv_d = nc.dram_tensor("wv", [C, HC * D], in_dt, kind="ExternalInput").ap()
    wp_d = nc.dram_tensor("wp", [HC * D, C], in_dt, kind="ExternalInput").ap()
    # bf16 proj partials: halves the output DMA (the tail's critical path);
    # the host accumulates the 4 partials per batch in f32.
    out_d = nc.dram_tensor("out", [N, C], out_dt, kind="ExternalOutput").ap()

    with tile.TileContext(nc) as tc:
        with (
            tc.tile_pool(name="const", bufs=1) as const,
            tc.tile_pool(name="ex", bufs=8) as expool,
            tc.tile_pool(name="den", bufs=6) as dpool,
            tc.tile_pool(name="stage", bufs=4) as stage,
            tc.tile_pool(name="stps", bufs=2, space="PSUM") as stps,
            tc.tile_pool(name="pvps", bufs=4, space="PSUM") as pvps,
        ):
            # persistent tiles
            # qkT chunks: 0 = q heads 0,1; 1 = q heads 2,3
            #   (head even -> partitions 0:64, odd -> 64:128)
            # kTp: per-head zero-padded K=128 stationary operand: head even
            #   has kT in rows 0:64 / zeros in 64:128, head odd the reverse,
            #   so a full-128-row matmul against the stacked q chunk
            #   contracts only the matching head's 64 features.
            qkT_sb = const.tile([128, 2, N], in_dt, tag="qkT")
            kTp_sb = const.tile([128, HC, N], in_dt, tag="kTp")
            v_sb = const.tile([128, 16, HC, D + 1], in_dt, tag="v")
            wp_sb = const.tile([128, 2, C], in_dt, tag="wp")
            outT_sb = const.tile([128, 2, N], in_dt, tag="outT")
            xT_sb = const.tile([128, KC, N], in_dt, tag="xT")
            wqk_sb = const.tile([128, KC, 2 * HC * D], in_dt, tag="wqk")
            wv_sb = const.tile([128, KC, HC * D], in_dt, tag="wv")

            # ---- DMAs, token-major for xT: wqk/xT(nt=0) interleaved so the
            # prologue qk chains start after ~0.5MB; attention block (0,0)
            # can then begin at ~6us instead of ~40us.
            # Inputs split across the Sync and GpSimd HW DMA queues so two
            # engines drain in parallel; the prologue-critical wqk+xT(nt=0)
            # chunks alternate queues kc-wise to land first.
            # The ~650ns/dma_start queue cost gates the prologue, so the 16
            # critical transfers (wqk + xT nt0) go 3-way across sync, gpsimd
            # AND the (otherwise idle at startup) scalar queue.
            qs = [nc.sync, nc.gpsimd, nc.scalar]

            def dma_xt(nt, q):
                for kc in range(KC):
                    q.dma_start(
                        xT_sb[:, kc, nt * 512 : (nt + 1) * 512],
                        xT_d[kc * 128 : (kc + 1) * 128, nt * 512 : (nt + 1) * 512],
                    )

            for kc in range(KC):
                qs[(2 * kc) % 3].dma_start(
                    wqk_sb[:, kc, :], wqk_d[kc * 128 : (kc + 1) * 128, :]
                )
                qs[(2 * kc + 1) % 3].dma_start(
                    xT_sb[:, kc, 0:512], xT_d[kc * 128 : (kc + 1) * 128, 0:512]
                )
            for kc in range(KC):
                qs[kc % 2].dma_start(wv_sb[:, kc, :], wv_d[kc * 128 : (kc + 1) * 128, :])
            dma_xt(1, nc.gpsimd)
            dma_xt(2, nc.sync)
            dma_xt(3, nc.gpsimd)
            for c2 in range(2):
                nc.sync.dma_start(wp_sb[:, c2, :], wp_d[c2 * 128 : (c2 + 1) * 128, :])

            # ---- one-time fills (run during the DMA wait) ----
            zsrc = const.tile([64, 512], f32, tag="zsrc")
            nc.vector.memset(zsrc[:], 0.0)
            for h in range(HC):
                zb = 64 if h % 2 == 0 else 0
                for nt in range(4):
                    nc.vector.tensor_copy(
                        kTp_sb[zb : zb + 64, h, nt * 512 : (nt + 1) * 512], zsrc[:]
                    )
            ones_f32 = const.tile([128, 16, HC, 1], f32, tag="ones")
            nc.vector.memset(ones_f32[:], 1.0)
            nc.vector.tensor_copy(v_sb[:, :, :, D : D + 1], ones_f32[:])

            # ---- emission helpers ----
            def qk_chunk(mf, nt):
                """One psum of (x @ Wqk)^T: feat chunk mf, token chunk nt.
                wqk feat chunks: 0 = q heads 0,1; 1 = q heads 2,3;
                2 = k heads 0,1; 3 = k heads 2,3."""
                ps = pvps.tile([128, 512], f32, tag="pv", name="pv")
                for kc in range(KC):
                    nc.tensor.matmul(
                        ps,
                        mm(wqk_sb[:, kc, mf * 128 : (mf + 1) * 128]),
                        mm(xT_sb[:, kc, nt * 512 : (nt + 1) * 512]),
                        start=(kc == 0),
                        stop=(kc == KC - 1),
                    )
                nts = slice(nt * 512, (nt + 1) * 512)
                if mf % 2 == 0:
                    mq = mf // 2
                    if mf < 2:
                        nc.vector.tensor_copy(qkT_sb[:, mq, nts], ps)
                    else:
                        pass
                if mf < 2:
                    if mf == 1:
                        nc.vector.tensor_copy(qkT_sb[:, 1, nts], ps)
                else:
                    h0, h1 = 2 * (mf - 2), 2 * (mf - 2) + 1
                    nc.vector.tensor_copy(kTp_sb[0:64, h0, nts], ps[0:64, :])
                    nc.vector.tensor_copy(kTp_sb[64:128, h1, nts], ps[64:128, :])

            def v_chunk(t):
                """One psum of v = x @ Wv for token(=key) chunk t, all heads."""
                ps = pvps.tile([128, 512], f32, tag="pv", name="pv")[:, : HC * D]
                for kc in range(KC):
                    nc.tensor.matmul(
                        ps,
                        mm(xT_sb[:, kc, t * 128 : (t + 1) * 128]),
                        mm(wv_sb[:, kc, :]),
                        start=(kc == 0),
                        stop=(kc == KC - 1),
                    )
                nc.vector.tensor_copy(
                    v_sb[:, t, :, 0:D], ps.rearrange("p (h d) -> p h d", h=HC)
                )

            sg2_of = {}

            def proj_chunk(t, nf):
                """partial[t*128:(t+1)*128, nf*512:(nf+1)*512] = out @ Wp.
                Both nf halves stage into one [128,1024] tile; the DMA (2KB
                rows, half the packets) fires once per token chunk."""
                ps = pvps.tile([128, 512], f32, tag="pv", name="pv")
                for c2 in range(2):
                    nc.tensor.matmul(
                        ps,
                        mm(outT_sb[:, c2, t * 128 : (t + 1) * 128]),
                        mm(wp_sb[:, c2, nf * 512 : (nf + 1) * 512]),
                        start=(c2 == 0),
                        stop=(c2 == 1),
                    )
                if nf == 0:
                    sg2_of[t] = stage.tile(
                        [128, 1024], out_dt, tag="sg2", name="sg2", bufs=2
                    )
                sg = sg2_of[t]
                nc.vector.tensor_copy(sg[:, nf * 512 : (nf + 1) * 512], ps)
                if nf == 1:
                    nc.sync.dma_start(out_d[t * 128 : (t + 1) * 128, :], sg)
                    del sg2_of[t]

            def proj_tail(t):
                """Both nf halves of token chunk t in one stps-pool psum
                (free after the last exp): fewer, wider tail ops + 2KB-row
                output DMA."""
                ps = stps.tile([128, 1024], f32, tag="st", name="st")
                for nf in range(2):
                    for c2 in range(2):
                        nc.tensor.matmul(
                            ps[:, nf * 512 : (nf + 1) * 512],
                            mm(outT_sb[:, c2, t * 128 : (t + 1) * 128]),
                            mm(wp_sb[:, c2, nf * 512 : (nf + 1) * 512]),
                            start=(c2 == 0),
                            stop=(c2 == 1),
                        )
                sg = stage.tile([128, 1024], out_dt, tag="sg2", name="sg2", bufs=2)
                nc.vector.tensor_copy(sg, ps)
                # tail runs after the last ACT, so the scalar queue is free
                (nc.sync if t % 2 == 0 else nc.scalar).dma_start(
                    out_d[t * 128 : (t + 1) * 128, :], sg
                )

            # fill queue: work interleaved into attention blocks' PE slack
            fills = []

            def attention_block(hp, rc, sched, tail=False):
                """ST + exp + PV for head pair hp, 512-row chunk rc; pops
                sched[kc2] fill closures at the top of each kc2 step."""
                heads = (2 * hp, 2 * hp + 1)
                pv = {
                    h: pvps.tile([128, 512], f32, tag="pv", name="pv") for h in heads
                }
                for kc2 in range(8):
                    for _ in range(sched[kc2]):
                        if fills:
                            fills.pop(0)()
                    stp = {
                        h: stps.tile([128, 1024], f32, tag="st", name="st")
                        for h in heads
                    }
                    for j in range(2):
                        kc = 2 * kc2 + j
                        for h in heads:
                            nc.tensor.matmul(
                                stp[h][:, j * 512 : (j + 1) * 512],
                                mm(kTp_sb[:, h, kc * 128 : (kc + 1) * 128]),
                                mm(qkT_sb[:, hp, rc * 512 : (rc + 1) * 512]),
                                start=True,
                                stop=True,
                            )
                    for h in heads:
                        ex = expool.tile([128, 1024], in_dt, tag="ex", name="ex")
                        nc.scalar.activation(
                            ex, stp[h], mybir.ActivationFunctionType.Exp, scale=SCALE
                        )
                        for j in range(2):
                            kc = 2 * kc2 + j
                            nc.tensor.matmul(
                                pv[h][: D + 1, :],
                                mm(v_sb[:, kc, h, :]),
                                mm(ex[:, j * 512 : (j + 1) * 512]),
                                start=(kc == 0),
                                stop=(kc == 15),
                            )
                if tail:
                    # the tail denominator chain leaves the PE idle ~3.7us --
                    # just over the HAM re-throttle window, which would halve
                    # the clock for the final proj matmuls. Dependency-free
                    # filler matmuls (f32 on zsrc, never consumed) bridge it.
                    wmt = stps.tile([128, 1024], f32, tag="st", name="wmt")
                    for _ in range(8):
                        nc.tensor.matmul(
                            wmt[:, 0:256],
                            zsrc[:, 0:128],
                            zsrc[:, 0:256],
                            start=True,
                            stop=True,
                        )
                # both recips first: DVE stays busy while GpSimd runs the
                # first broadcast, finishing the chain ~1us sooner (keeps the
                # post-block stall under the 3.4us HAM re-throttle window).
                # approx-fast recip: ~51 ULP, ~5x faster than the iterative
                # divide -- the denominator only needs ~1e-2 relative.
                dens, rbcs = {}, {}
                for h in heads:
                    dens[h] = dpool.tile([1, 512], f32, tag="den", name="den")
                    if os.environ.get("ATTN_RECIP", "fast") == "fast":
                        dsrc = dpool.tile([1, 512], f32, tag="dsrc", name="dsrc")
                        # tail: ScalarE is idle after the last exp -- staging
                        # the denominator there keeps the PE-idle gap under
                        # the ~3.4us HAM re-throttle window, so the tail proj
                        # matmuls run at full clock.
                        if tail:
                            nc.scalar.copy(dsrc, pv[h][D : D + 1, :])
                        else:
                            nc.vector.tensor_copy(dsrc, pv[h][D : D + 1, :])
                        nc.vector.reciprocal_approx_fast(out=dens[h], in_=dsrc)
                    else:
                        nc.vector.reciprocal(dens[h], pv[h][D : D + 1, :])
                for h in heads:
                    rbcs[h] = dpool.tile([64, 512], f32, tag="rbc", name="rbc")
                    nc.gpsimd.partition_broadcast(rbcs[h], dens[h])
                if not tail:
                    for h in heads:
                        hb = (h % 2) * 64
                        nc.vector.tensor_tensor(
                            out=outT_sb[hb : hb + 64, hp, rc * 512 : (rc + 1) * 512],
                            in0=pv[h][0:D, :],
                            in1=rbcs[h][:],
                            op=mybir.AluOpType.mult,
                        )
                else:
                    # final block: normalize per 128-token chunk and launch
                    # that chunk's out-proj + DMA immediately, so the tail
                    # pipeline (mult -> proj MM -> cast -> DMA) overlaps
                    # instead of serializing after the whole block.
                    for tc4 in range(4):
                        ts = slice(rc * 512 + tc4 * 128, rc * 512 + tc4 * 128 + 128)
                        for h in heads:
                            hb = (h % 2) * 64
                            nc.vector.tensor_tensor(
                                out=outT_sb[hb : hb + 64, hp, ts],
                                in0=pv[h][0:D, tc4 * 128 : (tc4 + 1) * 128],
                                in1=rbcs[h][:, tc4 * 128 : (tc4 + 1) * 128],
                                op=mybir.AluOpType.mult,
                            )
                        proj_tail(4 * rc + tc4)

            # ---- schedule ----
            def queue_proj(rc):
                fills.extend(
                    [
                        lambda t=t, nf=nf: proj_chunk(t, nf)
                        for t in range(4 * rc, 4 * rc + 4)
                        for nf in range(2)
                    ]
                )

            # Minimal prologue: exactly the chunks block (0,0) step 0 needs
            # (q heads01 rc0, kTp keys 0:256, v keys 0:256). Everything else
            # rides the fill queue inside the attention blocks, so ScalarE
            # exp starts at ~7us instead of ~40us and the PE is never
            # DMA-starved for long (HAM stays warm).
            qk_chunk(0, 0)
            qk_chunk(2, 0)
            v_chunk(0)
            v_chunk(1)
            # Fill order: v(2k),v(2k+1) land just before their PV(k) consumer
            # in block (0,0); kTp chunk qk(2,nt) before the step contracting
            # those keys; q chunks qk(0,1..3) before blocks (0,1..3).
            fills.extend(
                [
                    lambda: v_chunk(2),
                    lambda: v_chunk(3),
                    lambda: qk_chunk(2, 1),
                    lambda: v_chunk(4),
                    lambda: v_chunk(5),
                    lambda: v_chunk(6),
                    lambda: v_chunk(7),
                    lambda: qk_chunk(2, 2),
                    lambda: v_chunk(8),
                    lambda: v_chunk(9),
                    lambda: v_chunk(10),
                    lambda: v_chunk(11),
                    lambda: qk_chunk(2, 3),
                    lambda: v_chunk(12),
                    lambda: v_chunk(13),
                    lambda: qk_chunk(0, 1),
                    lambda: v_chunk(14),
                    lambda: v_chunk(15),
                ]
            )
            attention_block(0, 0, sched=[0, 3, 2, 3, 2, 3, 2, 3])
            fills.append(lambda: qk_chunk(0, 2))
            fills.append(lambda: qk_chunk(0, 3))
            fills.extend([lambda nt=nt: qk_chunk(1, nt) for nt in range(4)])
            fills.extend([lambda nt=nt: qk_chunk(3, nt) for nt in range(4)])
            # Alternating hp order: each (1,rc) block's denominator chain gets
            # a full unrelated block of latency before proj(rc) pops consume
            # its outT, so the PE never stalls on the recip/bcast/mult chain.
            late = [0, 1, 1, 1, 1, 1, 1, 1]
            attention_block(0, 1, sched=[2, 2, 2, 1, 1, 1, 1, 1])
            attention_block(1, 0, sched=[0] * 8)
            queue_proj(0)
            attention_block(0, 2, sched=[0] * 8)
            attention_block(1, 1, sched=late)
            queue_proj(1)
            attention_block(0, 3, sched=[0, 0, 0, 1, 1, 0, 0, 0])
            attention_block(1, 2, sched=late)
            queue_proj(2)
            attention_block(1, 3, sched=[0, 1, 1, 1, 1, 2, 1, 1], tail=True)
            # drain any straggler fills (none expected)
            while fills:
                fills.pop(0)()
    nc.compile()
    return nc


def make_in_maps(x, w_qkv, w_proj):
    in_maps = []
    for core in range(NCORES):
        b, g = core // 4, core % 4
        qs = slice(g * 256, (g + 1) * 256)
        in_maps.append(
            {
                "xT": _prep(x[b].T),
                "wqk": _prep(
                    np.concatenate(
                        [w_qkv[:, qs], w_qkv[:, C + g * 256 : C + (g + 1) * 256]],
                        axis=1,
                    )
                ),
                "wv": _prep(w_qkv[:, 2 * C + g * 256 : 2 * C + (g + 1) * 256]),
                "wp": _prep(w_proj[qs, :]),
            }
        )
    return in_maps


def run_hw(x, w_qkv, w_proj, b_proj, trace=False, tmpdir=None):
    """Returns (full output [2, 2048, 1024] f32, exec_time_ns or None)."""
    in_maps = make_in_maps(x, w_qkv, w_proj)
    nc = build_nc()
    r = run_bass_kernel_spmd(
        nc, in_maps, core_ids=list(range(NCORES)), trace=trace, tmpdir=tmpdir
    )
    full = np.zeros((B, N, C), np.float32)
    for core in range(NCORES):
        full[core // 4] += np.asarray(r.results[core]["out"], dtype=np.float32)
    full += np.asarray(b_proj, np.float32)[None, None, :]
    return full, r.exec_time_ns


def kernel(**inputs):
    x = np.asarray(inputs["x"], np.float32)
    w_qkv = np.asarray(inputs["w_qkv"], np.float32)
    w_proj = np.asarray(inputs["w_proj"], np.float32)
    b_proj = np.asarray(inputs["b_proj"], np.float32)
    out, _ = run_hw(x, w_qkv, w_proj, b_proj, trace=False)
    return out

